# revision 1
# baseline (speedup 1.0000x reference)
"""Trainium2 Bass kernel for nn_CANN_75857712382071.

Single-head self-attention (B=32, A=2048, D=128) with scalar output
projection, algebraically collapsed:

    out[b,aq] = (sum_ak E * (w+c+bo)) / (sum_ak E)
    E = exp(scale * (z M z^T + 1 (x) g)),  M = Wq^T Wk
    g[ak] = z[ak] . (Wk^T bq),   w[ak] = z[ak] . (Wv^T Wo^T)

q/k/v/h are never materialized; softmax max-subtraction is skipped
(logits are O(10); softmax is shift-invariant in exact arithmetic).

Data-parallel over batch: 4 batches per core on 8 NeuronCores.
Batches are software-pipelined: batch b+1's setup (z DMA, PE
transposes to zT, UT = M zT + gw, w column) is emitted in small pieces
inside batch b's main loop so ScalarE (exp) never starves.
"""

import sys
import types

import numpy as np

N_CORES = 8
B, A, D = 32, 2048, 128
B_PER = B // N_CORES
SCALE = float(D) ** -0.5
SCORES_DTYPE = "bf16"   # "bf16" (fast) or "f32r" (more precise scores)


def _install_axon_shim():
    """Allow run_bass_kernel_spmd(trace=True) to NTFF-profile under axon."""
    try:
        import antenv  # noqa: F401
    except ImportError:
        return
    if "antenv.axon_hooks" not in sys.modules:
        mod = types.ModuleType("antenv.axon_hooks")
        _hook = [None]
        mod.set_axon_ntff_profile_hook = lambda h: _hook.__setitem__(0, h)
        mod.get_axon_ntff_profile_hook = lambda: _hook[0]
        sys.modules["antenv.axon_hooks"] = mod
    from antenv.axon_hooks import (
        get_axon_ntff_profile_hook,
        set_axon_ntff_profile_hook,
    )
    if get_axon_ntff_profile_hook() is None:
        try:
            from trn_agent_boot.trn_boot import _ntff_profile_via_ctypes
            set_axon_ntff_profile_hook(
                _ntff_profile_via_ctypes("/opt/axon/libaxon_pjrt.so"))
        except Exception:
            pass
    try:
        from concourse import bass_utils
        bass_utils.upload_artifacts = lambda tmpdir: tmpdir
    except Exception:
        pass


def _build_program(cbo: float, scores_bf16: bool):
    import concourse.bacc as bacc
    import concourse.mybir as mybir
    import concourse.tile as tile
    from concourse import masks

    f32 = mybir.dt.float32
    f32r = mybir.dt.float32r
    bf16 = mybir.dt.bfloat16
    sdt = bf16 if scores_bf16 else f32r
    AF = mybir.ActivationFunctionType
    ADD = mybir.AluOpType.add
    MULT = mybir.AluOpType.mult

    nc = bacc.Bacc("TRN2", target_bir_lowering=False, debug=False,
                   num_devices=N_CORES, num_swdge_queues=2)

    z_d = nc.dram_tensor("z", [B_PER, A, D], f32, kind="ExternalInput").ap()
    m_d = nc.dram_tensor("m_lhs", [D, D], f32, kind="ExternalInput").ap()
    gw_d = nc.dram_tensor("gw", [D, 1], f32, kind="ExternalInput").ap()
    wv_d = nc.dram_tensor("wv", [D, 2], f32, kind="ExternalInput").ap()
    out_d = nc.dram_tensor("out", [B_PER, A], f32, kind="ExternalOutput").ap()

    NT = A // 128          # 16 ak tiles / z tiles
    NH = A // 1024         # 2 aq halves (ACT op width 1024)
    NC_ = A // 512         # 4 aq chunks (nd accumulators)

    with tile.TileContext(nc) as tc:
        with (
            tc.tile_pool(name="sb", bufs=1) as sb,
            tc.tile_pool(name="ps_sc", bufs=2, space="PSUM") as ps_sc,
        ):
            ps_nd = ps_sc
            sbc = sbz = sbe = sbb = sb
            # ---- constants ----
            m_f = sbc.tile([D, D], f32)
            nc.sync.dma_start(m_f[:], m_d[:])
            gw_col = sbc.tile([D, 1], f32)
            nc.sync.dma_start(gw_col[:], gw_d[:])
            wv_f = sbc.tile([D, 2], f32)
            nc.sync.dma_start(wv_f[:], wv_d[:])
            ident = sbc.tile([D, D], f32)
            masks.make_identity(nc, ident[:])
            m_r = sbc.tile([D, D], sdt)
            nc.vector.tensor_copy(m_r[:], m_f[:])
            wv_r = sbc.tile([D, 2], sdt)
            nc.vector.tensor_copy(wv_r[:], wv_f[:])

            # ACT table warmup (overlaps first z DMAs)
            warm = sbc.tile([D, 1], f32)
            nc.scalar.activation(warm[:], gw_col[:], AF.Exp, scale=0.0)

            # PE HAM pre-warm: dummy bf16 matmuls on junk data while the
            # first z DMA is in flight, so the prologue runs at 2.4 GHz
            junk = sbc.tile([128, 512], bf16)
            nc.gpsimd.memset(junk[:, 0:8], 0.0)
            pjunk = ps_sc.tile([128, 512], f32, name="pjunk", tag="sc")
            for i in range(14):
                nc.tensor.matmul(pjunk[:], junk[:, 0:128], junk[:],
                                 start=True, stop=True)

            st = {}  # per-batch live tiles

            def emit_z_dmas(b):
                s = st.setdefault(b, {})
                s["zT"] = sbb.tile([D, A], sdt, name=f"zT{b}", tag="zT", bufs=2)
                zn = sbz.tile([128, A], f32, name=f"zn{b}", tag="zn", bufs=2)
                zsrc = z_d[b].rearrange("(t p) d -> p t d", p=128)
                zdst = zn.rearrange("p (t d) -> p t d", d=D)
                engs = (nc.sync, nc.scalar, nc.gpsimd, nc.gpsimd)
                for q in range(8):
                    engs[q % 4].dma_start(zdst[:, 2 * q:2 * q + 2],
                                          zsrc[:, 2 * q:2 * q + 2])
                s["zn"] = zn

            def emit_cast_half(b, h):
                s = st[b]
                if "znb" not in s:
                    s["znb"] = sbz.tile([128, A], bf16, name=f"znb{b}",
                                        tag="znb", bufs=2)
                nc.vector.tensor_copy(s["znb"][:, h * 1024:(h + 1) * 1024],
                                      s["zn"][:, h * 1024:(h + 1) * 1024])

            def emit_transpose_dma(b, h):
                # 8 xbar DMA transposes straight into zT (no PE, no PSUM)
                s = st[b]
                for i in range(8 * h, 8 * h + 8):
                    eng = nc.sync if i % 2 == 0 else nc.scalar
                    eng.dma_start_transpose(
                        out=s["zT"][:, i * 128:(i + 1) * 128],
                        in_=s["znb"][:, i * 128:(i + 1) * 128])

            def emit_transpose_group(b, g, n=2):
                # n transposes into one psum slot, then 1 copy
                s = st[b]
                pt = ps_sc.tile([128, 1024], f32, name=f"pt{b}_{g}", tag="sc")
                for j in range(n):
                    i = n * g + j
                    nc.tensor.transpose(pt[:, j * 128:(j + 1) * 128],
                                        s["zn"][:, i * 128:(i + 1) * 128],
                                        ident[:])
                nc.vector.tensor_copy(
                    s["zT"][:, g * n * 128:(g + 1) * n * 128],
                    pt[:, 0:n * 128])

            def emit_ut_half(b, h):
                s = st[b]
                zT = s["zT"]
                if h == 0:
                    s["UT"] = sbb.tile([D, A], sdt, name=f"UT{b}", tag="UT",
                                       bufs=2)
                UT = s["UT"]
                pu = ps_sc.tile([128, 1024], f32, name=f"pu{b}_{h}",
                                tag="sc")
                for j in range(2):
                    o = h * 1024 + j * 512
                    nc.tensor.matmul(pu[:, j * 512:(j + 1) * 512],
                                     m_r[:], zT[:, o:o + 512],
                                     start=True, stop=True)
                nc.vector.tensor_scalar(
                    UT[:, h * 1024:(h + 1) * 1024], pu[:], gw_col[:],
                    None, ADD)

            def emit_w(b):
                s = st[b]
                zT = s["zT"]
                pw = ps_sc.tile([128, 2 * NT], f32, name=f"pw{b}", tag="sc")
                for t in range(NT):
                    nc.tensor.matmul(pw[:, 2 * t:2 * t + 2],
                                     zT[:, t * 128:(t + 1) * 128], wv_r[:],
                                     start=True, stop=True)
                wl = sbb.tile([128, 2 * NT], bf16, name=f"wl{b}", tag="wl",
                              bufs=2)
                nc.gpsimd.memset(wl[:], 1.0)
                wl3 = wl.rearrange("p (t two) -> p t two", two=2)
                pw3 = pw.rearrange("p (t two) -> p t two", two=2)
                nc.vector.tensor_scalar(wl3[:, :, 0], pw3[:, :, 0], cbo,
                                        None, ADD)
                s["wl"] = wl

            def emit_scores_tk(b, tk):
                s = st[b]
                lhs = s["zT"][:, tk * 128:(tk + 1) * 128]
                eTs = []
                for h in range(NH):
                    ps_t = ps_sc.tile([128, 1024], f32,
                                      name=f"s{b}_{tk}_{h}", tag="sc")
                    for j in range(2):
                        o = h * 1024 + j * 512
                        nc.tensor.matmul(ps_t[:, j * 512:(j + 1) * 512],
                                         lhs, s["UT"][:, o:o + 512],
                                         start=True, stop=True)
                    eT = sbe.tile([128, 1024], bf16,
                                  name=f"e{b}_{tk}_{h}", tag="eT", bufs=14)
                    nc.scalar.activation(eT[:], ps_t[:], AF.Exp, scale=SCALE)
                    eTs.append(eT)
                return eTs

            def emit_nd_tk(b, tk, eTs):
                s = st[b]
                wlt = s["wl"][:, 2 * tk:2 * tk + 2]
                for h in range(NH):
                    for j in range(2):
                        c = 2 * h + j
                        nc.tensor.matmul(
                            s["nd"][c][:], wlt,
                            eTs[h][:, j * 512:(j + 1) * 512],
                            start=(tk == 0), stop=(tk == NT - 1))

            def emit_finale(b):
                s = st[b]
                ndall = sbb.tile([2, A], f32, name=f"ndall{b}", tag="ndall", bufs=2)
                for c in range(NC_):
                    nc.vector.tensor_copy(
                        ndall[0:2, c * 512:(c + 1) * 512], s["nd"][c][:])
                # gather rows into [16,128]: partition t <- elements t*128..
                num16 = sbb.tile([16, 128], f32, name=f"num16{b}", tag="num16", bufs=2)
                den16 = sbb.tile([16, 128], f32, name=f"den16{b}", tag="den16", bufs=2)
                nc.scalar.dma_start(
                    den16[:, :],
                    ndall[1:2, :].rearrange("one (t p) -> one t p", p=128))
                nc.sync.dma_start(
                    num16[:, :],
                    ndall[0:1, :].rearrange("one (t p) -> one t p", p=128))
                rcp = sbb.tile([16, 128], f32, name=f"rcp{b}", tag="rcp", bufs=2)
                nc.vector.reciprocal(rcp[:], den16[:])
                o16 = sbb.tile([16, 128], f32, name=f"o16{b}", tag="o16", bufs=2)
                nc.vector.tensor_tensor(o16[:], num16[:], rcp[:], MULT)
                nc.sync.dma_start(
                    out_d[b:b + 1, :].rearrange("one (t p) -> one t p", p=128),
                    o16[:, :])
                st.pop(b)

            # ---- prologue: batch 0 setup (lean critical path) ----
            emit_z_dmas(0)
            emit_transpose_group(0, 0, n=4)
            emit_transpose_group(0, 1, n=4)
            emit_ut_half(0, 0)
            emit_transpose_group(0, 2, n=4)
            emit_transpose_group(0, 3, n=4)
            emit_ut_half(0, 1)
            emit_w(0)
            del st[0]["zn"]

            pend = []            # [(b, tk, eTs)] awaiting nd emission
            def flush_pend(keep):
                while len(pend) > keep:
                    pb, ptk, peTs = pend.pop(0)
                    emit_nd_tk(pb, ptk, peTs)
                    if ptk == NT - 1:
                        emit_finale(pb)
            for b in range(B_PER):
                s = st[b]
                s["nd"] = [ps_nd.tile([2, 512], f32, name=f"nd{b}_{c}",
                                      tag=f"nd{c}", bufs=1)
                           for c in range(NC_)]
                nxt = b + 1 if b + 1 < B_PER else None
                for tk in range(NT):
                    eTs = emit_scores_tk(b, tk)
                    if len(pend) >= (2 if b == B_PER - 1 else 4):
                        flush_pend(0)
                    pend.append((b, tk, eTs))
                    if nxt is not None:
                        if tk == 1:
                            emit_z_dmas(nxt)
                        elif tk in (5, 7, 9, 11):
                            emit_transpose_group(nxt, (tk - 5) // 2, n=4)
                        elif tk == 12:
                            emit_ut_half(nxt, 0)
                        elif tk == 13:
                            emit_ut_half(nxt, 1)
                        elif tk == 14:
                            emit_w(nxt)
            flush_pend(0)

    nc.compile()
    return nc


def run(inputs: dict, trace: bool = False):
    _install_axon_shim()
    from concourse.bass_utils import run_bass_kernel_spmd

    z = np.asarray(inputs["z"], dtype=np.float32)
    Wq = np.asarray(inputs["Wq"], dtype=np.float64)
    bq = np.asarray(inputs["bq"], dtype=np.float64)
    Wk = np.asarray(inputs["Wk"], dtype=np.float64)
    Wv = np.asarray(inputs["Wv"], dtype=np.float64)
    bv = np.asarray(inputs["bv"], dtype=np.float64)
    Wo = np.asarray(inputs["Wo"], dtype=np.float64)
    bo = np.asarray(inputs["bo"], dtype=np.float64)

    # host-side weight algebra (tiny, exact in float64)
    m_lhs = (Wq.T @ Wk).astype(np.float32)            # [d, d']
    gw = (Wk.T @ bq).astype(np.float32).reshape(D, 1)
    wv = np.repeat((Wv.T @ Wo[0]).astype(np.float32).reshape(D, 1), 2, axis=1)
    cbo = float(bv @ Wo[0] + bo[0])

    nc = _build_program(cbo, SCORES_DTYPE == "bf16")

    in_maps = []
    for c in range(N_CORES):
        in_maps.append({
            "z": z[c * B_PER:(c + 1) * B_PER],
            "m_lhs": m_lhs,
            "gw": gw,
            "wv": wv,
        })
    res = run_bass_kernel_spmd(nc, in_maps, core_ids=list(range(N_CORES)),
                               trace=trace)
    out = np.concatenate([res.results[c]["out"] for c in range(N_CORES)],
                         axis=0)
    return out.reshape(B, A, 1).astype(np.float32), res


def kernel(**inputs) -> np.ndarray:
    out, _ = run(inputs, trace=False)
    return out



# revision 6
# speedup vs baseline: 1.0669x; 1.0669x over previous
"""Trainium2 Bass kernel for nn_CANN_75857712382071.

Single-head self-attention (B=32, A=2048, D=128) with scalar output
projection, algebraically collapsed:

    out[b,aq] = (sum_ak E * (w+c+bo)) / (sum_ak E)
    E = exp(scale * (z M z^T + 1 (x) g)),  M = Wq^T Wk
    g[ak] = z[ak] . (Wk^T bq),   w[ak] = z[ak] . (Wv^T Wo^T)

q/k/v/h are never materialized; softmax max-subtraction is skipped
(logits are O(10); softmax is shift-invariant in exact arithmetic).

Data-parallel over batch: 4 batches per core on 8 NeuronCores.

Schedule: ScalarE (exp, 1100ns per 1024-wide tile) is the bottleneck
engine, so everything else is arranged to keep it fed:
  - aq-half-major loop => nd accumulators need only 2 PSUM banks,
    leaving 6 banks = 3 rotating [128,1024] score slots (deep exp
    pipeline instead of 2-slot ping-pong).
  - z arrives as bf16 (host cast), transposed by ONE xbar-DMA
    instruction per batch (3D out AP) -- no PE transposes, no PSUM.
  - w column via DVE tensor_tensor_reduce from zn -- no PE matmuls.
  - all DMA issues on sync/vector/gpsimd queues, never on ScalarE.
  - batch b+1 setup (DMA, transpose, w, UT) interleaved into b's loop.
"""

import sys
import types

import numpy as np

N_CORES = 8
B, A, D = 32, 2048, 128
B_PER = B // N_CORES
SCALE = float(D) ** -0.5


def _install_axon_shim():
    """Allow run_bass_kernel_spmd(trace=True) to NTFF-profile under axon."""
    try:
        import antenv  # noqa: F401
    except ImportError:
        return
    if "antenv.axon_hooks" not in sys.modules:
        mod = types.ModuleType("antenv.axon_hooks")
        _hook = [None]
        mod.set_axon_ntff_profile_hook = lambda h: _hook.__setitem__(0, h)
        mod.get_axon_ntff_profile_hook = lambda: _hook[0]
        sys.modules["antenv.axon_hooks"] = mod
    from antenv.axon_hooks import (
        get_axon_ntff_profile_hook,
        set_axon_ntff_profile_hook,
    )
    if get_axon_ntff_profile_hook() is None:
        try:
            from trn_agent_boot.trn_boot import _ntff_profile_via_ctypes
            set_axon_ntff_profile_hook(
                _ntff_profile_via_ctypes("/opt/axon/libaxon_pjrt.so"))
        except Exception:
            pass
    try:
        from concourse import bass_utils
        bass_utils.upload_artifacts = lambda tmpdir: tmpdir
    except Exception:
        pass


def _build_program(cbo: float):
    import os
    use_gpd = os.environ.get("K_GPD", "1") == "1"
    import concourse.bacc as bacc
    import concourse.mybir as mybir
    import concourse.tile as tile

    f32 = mybir.dt.float32
    bf16 = mybir.dt.bfloat16
    AF = mybir.ActivationFunctionType
    ADD = mybir.AluOpType.add
    MULT = mybir.AluOpType.mult

    nc = bacc.Bacc("TRN2", target_bir_lowering=False, debug=False,
                   num_devices=N_CORES, num_swdge_queues=2)

    z_d = nc.dram_tensor("z", [B_PER, A, D], bf16, kind="ExternalInput").ap()
    m_d = nc.dram_tensor("m_lhs", [D, D], f32, kind="ExternalInput").ap()
    gw_d = nc.dram_tensor("gw", [D, 1], f32, kind="ExternalInput").ap()
    wvb_d = nc.dram_tensor("wvb", [128, A], bf16, kind="ExternalInput").ap()
    out_d = nc.dram_tensor("out", [B_PER, A], f32, kind="ExternalOutput").ap()

    NT = A // 128          # 16 key tiles
    NH = 2                 # aq halves (1024 each)

    with tile.TileContext(nc) as tc:
        with (
            tc.tile_pool(name="sb", bufs=1) as sb,
            tc.tile_pool(name="ps", bufs=3, space="PSUM") as ps,
        ):
            # ---- constants ----
            m_f = sb.tile([D, D], f32)
            nc.sync.dma_start(m_f[:], m_d[:])
            gw_col = sb.tile([D, 1], f32)
            nc.sync.dma_start(gw_col[:], gw_d[:])
            wvb = sb.tile([128, A], bf16)
            nc.sync.dma_start(wvb[:], wvb_d[:])
            m_r = sb.tile([D, D], bf16)
            nc.vector.tensor_copy(m_r[:], m_f[:])

            # ACT table warmup (overlaps first z DMAs)
            warm = sb.tile([D, 1], f32)
            nc.scalar.activation(warm[:], gw_col[:], AF.Exp, scale=0.0)

            # PE HAM pre-warm: dummy bf16 matmuls on junk data while the
            # first z DMA is in flight, so the prologue runs at 2.4 GHz
            junk = sb.tile([128, 512], bf16)
            nc.gpsimd.memset(junk[:], 0.0)
            pjunk = ps.tile([128, 512], f32, name="pjunk", tag="sc")
            for i in range(14):
                nc.tensor.matmul(pjunk[:], junk[:, 0:128], junk[:],
                                 start=True, stop=True)

            st = {}  # per-batch live tiles

            def emit_z_dmas(b, grp):
                # grp 0/1: four 2-tile chunks each
                s = st.setdefault(b, {})
                if grp == 0:
                    s["zn"] = sb.tile([128, A], bf16, name=f"zn{b}",
                                      tag="zn", bufs=2)
                zn = s["zn"]
                zsrc = z_d[b].rearrange("(t p) d -> p t d", p=128)
                zdst = zn.rearrange("p (t d) -> p t d", d=D)
                engs = ((nc.sync, nc.gpsimd, nc.sync, nc.gpsimd)
                        if use_gpd else
                        (nc.sync, nc.sync, nc.sync, nc.sync))
                for q in range(4 * grp, 4 * grp + 4):
                    engs[q % 4].dma_start(zdst[:, 2 * q:2 * q + 2],
                                          zsrc[:, 2 * q:2 * q + 2])

            def emit_transpose(b):
                # ONE xbar DMA: per-128-block transpose zn -> zT
                s = st[b]
                s["zT"] = sb.tile([D, A], bf16, name=f"zT{b}", tag="zT",
                                  bufs=2)
                nc.sync.dma_start_transpose(
                    out=s["zT"].rearrange("p (t q) -> p t q", q=128),
                    in_=s["zn"][:])

            def emit_w_ttr(b, step):
                # w[ak] = z[ak].wv: DVE mult into scratch, reduce over d
                s = st[b]
                if step == 0:
                    s["wacc"] = sb.tile([128, NT], f32, name=f"wacc{b}",
                                        tag="wacc", bufs=2)
                    s["scr"] = sb.tile([128, A], bf16, name=f"scr{b}",
                                       tag="scr", bufs=2)
                    nc.vector.tensor_tensor(s["scr"][:], s["zn"][:],
                                            wvb[:], MULT)
                else:
                    scr3 = s["scr"].rearrange("p (t d) -> p t d", d=D)
                    nc.vector.tensor_reduce(
                        s["wacc"][:], scr3[:], axis=mybir.AxisListType.X,
                        op=ADD)

            def emit_wl(b):
                s = st[b]
                wl = sb.tile([128, 2 * NT], bf16, name=f"wl{b}", tag="wl",
                             bufs=2)
                nc.gpsimd.memset(wl[:], 1.0)
                wl3 = wl.rearrange("p (t two) -> p t two", two=2)
                nc.vector.tensor_scalar(wl3[:, :, 0], s["wacc"][:], cbo,
                                        None, ADD)
                s["wl"] = wl

            def emit_ut_half(b, h):
                s = st[b]
                if h == 0:
                    s["UT"] = sb.tile([D, A], bf16, name=f"UT{b}", tag="UT",
                                      bufs=2)
                pu = ps.tile([128, 1024], f32, name=f"pu{b}_{h}", tag="sc")
                for j in range(2):
                    o = h * 1024 + j * 512
                    nc.tensor.matmul(pu[:, j * 512:(j + 1) * 512],
                                     m_r[:], s["zT"][:, o:o + 512],
                                     start=True, stop=True)
                nc.vector.tensor_scalar(
                    s["UT"][:, h * 1024:(h + 1) * 1024], pu[:], gw_col[:],
                    None, ADD)

            def emit_scores(b, h, tk):
                s = st[b]
                lhs = s["zT"][:, tk * 128:(tk + 1) * 128]
                ps_t = ps.tile([128, 1024], f32, name=f"s{b}_{h}_{tk}",
                               tag="sc")
                for j in range(2):
                    o = h * 1024 + j * 512
                    nc.tensor.matmul(ps_t[:, j * 512:(j + 1) * 512],
                                     lhs, s["UT"][:, o:o + 512],
                                     start=True, stop=True)
                eT = sb.tile([128, 1024], bf16, name=f"e{b}_{h}_{tk}",
                             tag="eT", bufs=8)
                nc.scalar.activation(eT[:], ps_t[:], AF.Exp, scale=SCALE)
                return eT

            def emit_nd(b, h, tk, eT):
                s = st[b]
                wlt = s["wl"][:, 2 * tk:2 * tk + 2]
                for c in range(2):
                    nc.tensor.matmul(
                        s["nd"][c][:], wlt, eT[:, c * 512:(c + 1) * 512],
                        start=(tk == 0), stop=(tk == NT - 1))

            def emit_nd_flush(b, h):
                # nd PSUM -> ndall SBUF for this half
                s = st[b]
                for c in range(2):
                    o = h * 1024 + c * 512
                    nc.vector.tensor_copy(s["ndall"][0:2, o:o + 512],
                                          s["nd"][c][:])

            def emit_finale(b):
                s = st[b]
                num16 = sb.tile([16, 128], f32, name=f"num16{b}",
                                tag="num16", bufs=2)
                den16 = sb.tile([16, 128], f32, name=f"den16{b}",
                                tag="den16", bufs=2)
                nc.sync.dma_start(
                    num16[:, :],
                    s["ndall"][0:1, :].rearrange("one (t p) -> one t p",
                                                 p=128))
                (nc.gpsimd if use_gpd else nc.sync).dma_start(
                    den16[:, :],
                    s["ndall"][1:2, :].rearrange("one (t p) -> one t p",
                                                 p=128))
                rcp = sb.tile([16, 128], f32, name=f"rcp{b}", tag="rcp",
                              bufs=2)
                nc.vector.reciprocal(rcp[:], den16[:])
                o16 = sb.tile([16, 128], f32, name=f"o16{b}", tag="o16",
                              bufs=2)
                nc.vector.tensor_tensor(o16[:], num16[:], rcp[:], MULT)
                (nc.gpsimd if use_gpd else nc.sync).dma_start(
                    out_d[b:b + 1, :].rearrange("one (t p) -> one t p",
                                                p=128),
                    o16[:, :])
                st.pop(b)

            # ---- prologue: batch 0 setup ----
            emit_z_dmas(0, 0)
            emit_z_dmas(0, 1)
            emit_transpose(0)
            emit_w_ttr(0, 0)
            emit_w_ttr(0, 1)
            emit_wl(0)
            emit_ut_half(0, 0)
            emit_ut_half(0, 1)

            # setup pieces for the NEXT batch, keyed by (h, tk) of the
            # current batch's main loop
            def setup_piece(nxt, h, tk):
                if h == 0:
                    if tk == 1:
                        emit_z_dmas(nxt, 0)
                    elif tk == 3:
                        emit_z_dmas(nxt, 1)
                    elif tk == 12:
                        emit_transpose(nxt)
                else:
                    if tk in (0, 4):
                        emit_w_ttr(nxt, tk // 4)
                    elif tk == 8:
                        emit_wl(nxt)
                    elif tk == 10:
                        emit_ut_half(nxt, 0)
                    elif tk == 12:
                        emit_ut_half(nxt, 1)

            for b in range(B_PER):
                s = st[b]
                s["ndall"] = sb.tile([2, A], f32, name=f"ndall{b}",
                                     tag="ndall", bufs=2)
                nxt = b + 1 if b + 1 < B_PER else None
                for h in range(NH):
                    s["nd"] = [ps.tile([2, 512], f32, name=f"nd{b}_{h}_{c}",
                                       tag=f"nd{c}", bufs=1)
                               for c in range(2)]
                    pend = []
                    for tk in range(NT):
                        eT = emit_scores(b, h, tk)
                        pend.append((tk, eT))
                        keep = 3 if tk < 13 else 15 - tk
                        while len(pend) > keep:
                            ptk, peT = pend.pop(0)
                            emit_nd(b, h, ptk, peT)
                        if nxt is not None:
                            setup_piece(nxt, h, tk)
                    emit_nd_flush(b, h)
                emit_finale(b)

    nc.compile()
    return nc


def run(inputs: dict, trace: bool = False):
    _install_axon_shim()
    import ml_dtypes
    from concourse.bass_utils import run_bass_kernel_spmd

    z = np.asarray(inputs["z"], dtype=np.float32)
    Wq = np.asarray(inputs["Wq"], dtype=np.float64)
    bq = np.asarray(inputs["bq"], dtype=np.float64)
    Wk = np.asarray(inputs["Wk"], dtype=np.float64)
    Wv = np.asarray(inputs["Wv"], dtype=np.float64)
    bv = np.asarray(inputs["bv"], dtype=np.float64)
    Wo = np.asarray(inputs["Wo"], dtype=np.float64)
    bo = np.asarray(inputs["bo"], dtype=np.float64)

    # host-side weight algebra (tiny, exact in float64)
    m_lhs = (Wq.T @ Wk).astype(np.float32)            # [d, d']
    gw = (Wk.T @ bq).astype(np.float32).reshape(D, 1)
    wv = (Wv.T @ Wo[0]).astype(np.float32)            # [d]
    wvb = np.broadcast_to(np.tile(wv, A // D), (128, A)).astype(
        ml_dtypes.bfloat16)
    cbo = float(bv @ Wo[0] + bo[0])

    z_bf = z.astype(ml_dtypes.bfloat16)

    nc = _build_program(cbo)

    in_maps = []
    for c in range(N_CORES):
        in_maps.append({
            "z": z_bf[c * B_PER:(c + 1) * B_PER],
            "m_lhs": m_lhs,
            "gw": gw,
            "wvb": np.ascontiguousarray(wvb),
        })
    res = run_bass_kernel_spmd(nc, in_maps, core_ids=list(range(N_CORES)),
                               trace=trace)
    out = np.concatenate([res.results[c]["out"] for c in range(N_CORES)],
                         axis=0)
    return out.reshape(B, A, 1).astype(np.float32), res


def kernel(**inputs) -> np.ndarray:
    out, _ = run(inputs, trace=False)
    return out


# revision 7
# speedup vs baseline: 1.1978x; 1.1227x over previous
"""Trainium2 Bass kernel for nn_CANN_75857712382071.

Single-head self-attention (B=32, A=2048, D=128) with scalar output
projection, algebraically collapsed:

    out[b,aq] = (sum_ak E * (w+c+bo)) / (sum_ak E)
    E = exp(scale * (z M z^T + 1 (x) g)),  M = Wq^T Wk
    g[ak] = z[ak] . (Wk^T bq),   w[ak] = z[ak] . (Wv^T Wo^T)

q/k/v/h are never materialized; softmax max-subtraction is skipped
(logits are O(10); softmax is shift-invariant in exact arithmetic).

Data-parallel over batch: 4 batches per core on 8 NeuronCores.

Engine budget per batch (measured): PE 33us (128 512-wide matmuls:
scores + num/den reduction), ScalarE 36us if it does all 32 exps.
To balance, 4 of 32 exp tiles per batch go to DVE using a Schraudolph
bf16 bit-trick (int16(A*s+B) bitcast as bf16 ~= exp(scale*s), C
calibrated for truncation); DVE reads PSUM directly (GpSimd cannot).

Schedule highlights:
  - aq-half-major loop: nd accumulators need 1-2 PSUM banks -> 3
    rotating [128,1024] score slots + dedicated UT-quarter bank.
  - zT via 2 DRAM-direct xbar-DMA transposes per batch (z uploaded
    bf16); zn staged separately only for the w column (DVE).
  - bulk DMA on the sync HWDGE queue (16-engine fan-out); nothing on
    ScalarE's queue.
  - single combined output DMA at the end (4KB teardown-cheap).
  - batch b+1 setup interleaved into b's loop; finale of b-1 inside b.
"""

import sys
import types

import numpy as np

N_CORES = 8
B, A, D = 32, 2048, 128
B_PER = B // N_CORES
SCALE = float(D) ** -0.5

SCHR_TKS = (3, 7, 11, 15)     # exp tiles handled by DVE bit-trick
SCHR_C = 6.0
ND1BANK = True                # nd chunks packed into one PSUM bank


def _install_axon_shim():
    """Allow run_bass_kernel_spmd(trace=True) to NTFF-profile under axon."""
    try:
        import antenv  # noqa: F401
    except ImportError:
        return
    if "antenv.axon_hooks" not in sys.modules:
        mod = types.ModuleType("antenv.axon_hooks")
        _hook = [None]
        mod.set_axon_ntff_profile_hook = lambda h: _hook.__setitem__(0, h)
        mod.get_axon_ntff_profile_hook = lambda: _hook[0]
        sys.modules["antenv.axon_hooks"] = mod
    from antenv.axon_hooks import (
        get_axon_ntff_profile_hook,
        set_axon_ntff_profile_hook,
    )
    if get_axon_ntff_profile_hook() is None:
        try:
            from trn_agent_boot.trn_boot import _ntff_profile_via_ctypes
            set_axon_ntff_profile_hook(
                _ntff_profile_via_ctypes("/opt/axon/libaxon_pjrt.so"))
        except Exception:
            pass
    try:
        from concourse import bass_utils
        bass_utils.upload_artifacts = lambda tmpdir: tmpdir
    except Exception:
        pass


def _build_program(cbo: float):
    import concourse.bacc as bacc
    import concourse.mybir as mybir
    import concourse.tile as tile

    f32 = mybir.dt.float32
    bf16 = mybir.dt.bfloat16
    i16 = mybir.dt.int16
    AF = mybir.ActivationFunctionType
    ADD = mybir.AluOpType.add
    MULT = mybir.AluOpType.mult

    A16S = float(128.0 / np.log(2.0) * SCALE)
    B16S = float(127.0 * 128.0 - SCHR_C)

    nc = bacc.Bacc("TRN2", target_bir_lowering=False, debug=False,
                   num_devices=N_CORES, num_swdge_queues=2)

    z_d = nc.dram_tensor("z", [B_PER, A, D], bf16, kind="ExternalInput").ap()
    m_d = nc.dram_tensor("m_lhs", [D, D], f32, kind="ExternalInput").ap()
    gw_d = nc.dram_tensor("gw", [D, 1], f32, kind="ExternalInput").ap()
    wvb_d = nc.dram_tensor("wvb", [128, A], bf16, kind="ExternalInput").ap()
    out_d = nc.dram_tensor("out", [B_PER, A], f32, kind="ExternalOutput").ap()

    NT = A // 128          # 16 key tiles
    NH = 2                 # aq halves (1024 each)

    with tile.TileContext(nc) as tc:
        with (
            tc.tile_pool(name="sb", bufs=1) as sb,
            tc.tile_pool(name="ps", bufs=3, space="PSUM") as ps,
        ):
            # ---- constants ----
            m_f = sb.tile([D, D], f32)
            nc.sync.dma_start(m_f[:], m_d[:])
            gw_col = sb.tile([D, 1], f32)
            nc.sync.dma_start(gw_col[:], gw_d[:])
            wvb = sb.tile([128, A], bf16)
            nc.sync.dma_start(wvb[:], wvb_d[:])
            m_r = sb.tile([D, D], bf16)
            nc.vector.tensor_copy(m_r[:], m_f[:])

            # ACT table warmup (overlaps first z DMAs)
            warm = sb.tile([D, 1], f32)
            nc.scalar.activation(warm[:], gw_col[:], AF.Exp, scale=0.0)

            # PE HAM pre-warm on junk data while the first DMAs fly
            junk = sb.tile([128, 512], bf16)
            nc.gpsimd.memset(junk[:], 0.0)
            pjunk = ps.tile([128, 512], f32, name="pjunk", tag="sc")
            for i in range(14):
                nc.tensor.matmul(pjunk[:], junk[:, 0:128], junk[:],
                                 start=True, stop=True)

            st = {}  # per-batch live tiles

            def emit_transpose(b, h):
                # DRAM-direct xbar transpose of one aq half into zT
                s = st.setdefault(b, {})
                if h == 0:
                    s["zT"] = sb.tile([D, A], bf16, name=f"zT{b}", tag="zT",
                                      bufs=2)
                nc.sync.dma_start_transpose(
                    out=s["zT"][:, h * 1024:(h + 1) * 1024],
                    in_=z_d[b][h * 1024:(h + 1) * 1024, :])

            def emit_z_dma(b, grp):
                # zn staging (w column only): 2 fat chunks on sync HWDGE
                s = st[b]
                if grp == 0:
                    s["zn"] = sb.tile([128, A], bf16, name=f"zn{b}",
                                      tag="zn", bufs=2)
                zsrc = z_d[b].rearrange("(t p) d -> p t d", p=128)
                zdst = s["zn"].rearrange("p (t d) -> p t d", d=D)
                nc.sync.dma_start(zdst[:, 8 * grp:8 * grp + 8],
                                  zsrc[:, 8 * grp:8 * grp + 8])

            def emit_w_mult(b):
                s = st[b]
                s["scr"] = sb.tile([128, A], bf16, name=f"scr{b}",
                                   tag="scr", bufs=2)
                nc.vector.tensor_tensor(s["scr"][:], s["zn"][:], wvb[:],
                                        MULT)

            def emit_w_reduce(b):
                s = st[b]
                s["wacc"] = sb.tile([128, NT], f32, name=f"wacc{b}",
                                    tag="wacc", bufs=2)
                scr3 = s["scr"].rearrange("p (t d) -> p t d", d=D)
                nc.vector.tensor_reduce(
                    s["wacc"][:], scr3[:], axis=mybir.AxisListType.X,
                    op=ADD)

            def emit_wl(b):
                s = st[b]
                wl = sb.tile([128, 2 * NT], bf16, name=f"wl{b}", tag="wl",
                             bufs=2)
                nc.gpsimd.memset(wl[:], 1.0)
                wl3 = wl.rearrange("p (t two) -> p t two", two=2)
                nc.vector.tensor_scalar(wl3[:, :, 0], s["wacc"][:], cbo,
                                        None, ADD)
                s["wl"] = wl

            def emit_ut_quarter(b, q):
                s = st[b]
                if q == 0:
                    s["UT"] = sb.tile([D, A], bf16, name=f"UT{b}", tag="UT",
                                      bufs=2)
                pu = ps.tile([128, 512], f32, name=f"pu{b}_{q}", tag="pu",
                             bufs=1)
                o = q * 512
                nc.tensor.matmul(pu[:], m_r[:], s["zT"][:, o:o + 512],
                                 start=True, stop=True)
                nc.vector.tensor_scalar(s["UT"][:, o:o + 512], pu[:],
                                        gw_col[:], None, ADD)

            def emit_scores(b, h, tk):
                s = st[b]
                lhs = s["zT"][:, tk * 128:(tk + 1) * 128]
                ps_t = ps.tile([128, 1024], f32, name=f"s{b}_{h}_{tk}",
                               tag="sc")
                for j in range(2):
                    o = h * 1024 + j * 512
                    nc.tensor.matmul(ps_t[:, j * 512:(j + 1) * 512],
                                     lhs, s["UT"][:, o:o + 512],
                                     start=True, stop=True)
                eT = sb.tile([128, 1024], bf16, name=f"e{b}_{h}_{tk}",
                             tag="eT", bufs=8)
                if tk in SCHR_TKS:
                    nc.vector.tensor_scalar(eT.bitcast(i16)[:], ps_t[:],
                                            A16S, B16S, MULT, ADD)
                else:
                    nc.scalar.activation(eT[:], ps_t[:], AF.Exp,
                                         scale=SCALE)
                return eT

            def emit_nd(b, h, tk, eT):
                s = st[b]
                wlt = s["wl"][:, 2 * tk:2 * tk + 2]
                for c in range(2):
                    nc.tensor.matmul(
                        s["nd"][c], wlt, eT[:, c * 512:(c + 1) * 512],
                        start=(tk == 0), stop=(tk == NT - 1))

            def alloc_nd(b, h):
                s = st[b]
                if ND1BANK:
                    ndt = ps.tile([34, 512], f32, name=f"nd{b}_{h}",
                                  tag="nd", bufs=1)
                    s["nd"] = [ndt[0:2, :], ndt[32:34, :]]
                else:
                    s["nd"] = [ps.tile([2, 512], f32, name=f"nd{b}_{h}_{c}",
                                       tag=f"nd{c}", bufs=1)[:]
                               for c in range(2)]

            def emit_nd_flush(b, h):
                s = st[b]
                for c in range(2):
                    o = h * 1024 + c * 512
                    nc.vector.tensor_copy(s["ndall"][0:2, o:o + 512],
                                          s["nd"][c])

            o16 = sb.tile([16, B_PER * 128], f32, name="o16")

            def emit_finale(b, step):
                s = st[b]
                if step == 0:
                    s["num16"] = sb.tile([16, 128], f32, name=f"num16{b}",
                                         tag="num16", bufs=2)
                    nc.sync.dma_start(
                        s["num16"][:, :],
                        s["ndall"][0:1, :].rearrange(
                            "one (t p) -> one t p", p=128))
                elif step == 1:
                    s["den16"] = sb.tile([16, 128], f32, name=f"den16{b}",
                                         tag="den16", bufs=2)
                    nc.sync.dma_start(
                        s["den16"][:, :],
                        s["ndall"][1:2, :].rearrange(
                            "one (t p) -> one t p", p=128))
                elif step == 2:
                    s["rcp"] = sb.tile([16, 128], f32, name=f"rcp{b}",
                                       tag="rcp", bufs=2)
                    nc.vector.reciprocal(s["rcp"][:], s["den16"][:])
                else:
                    nc.vector.tensor_tensor(
                        o16[:, b * 128:(b + 1) * 128], s["num16"][:],
                        s["rcp"][:], MULT)
                    st.pop(b)

            # ---- prologue: batch 0 setup ----
            emit_transpose(0, 0)
            emit_transpose(0, 1)
            emit_z_dma(0, 0)
            emit_z_dma(0, 1)
            emit_w_mult(0)
            emit_w_reduce(0)
            emit_wl(0)
            for q in range(4):
                emit_ut_quarter(0, q)

            def setup_piece(b, nxt, h, tk):
                # finale of b-1 early in b; setup of b+1 spread through b
                if h == 0 and tk in (1, 2, 3, 4) and (b - 1) in st:
                    emit_finale(b - 1, tk - 1)
                if nxt is None:
                    return
                if h == 0:
                    if tk == 5:
                        emit_transpose(nxt, 0)
                    elif tk == 7:
                        emit_transpose(nxt, 1)
                    elif tk == 9:
                        emit_z_dma(nxt, 0)
                    elif tk == 11:
                        emit_z_dma(nxt, 1)
                else:
                    if tk == 0:
                        emit_w_mult(nxt)
                    elif tk == 2:
                        emit_w_reduce(nxt)
                    elif tk == 4:
                        emit_wl(nxt)
                    elif tk in (6, 8, 10, 12):
                        emit_ut_quarter(nxt, (tk - 6) // 2)

            for b in range(B_PER):
                s = st[b]
                s["ndall"] = sb.tile([2, A], f32, name=f"ndall{b}",
                                     tag="ndall", bufs=2)
                nxt = b + 1 if b + 1 < B_PER else None
                for h in range(NH):
                    alloc_nd(b, h)
                    pend = []
                    for tk in range(NT):
                        eT = emit_scores(b, h, tk)
                        pend.append((tk, eT))
                        keep = 3 if tk < 13 else 15 - tk
                        while len(pend) > keep:
                            ptk, peT = pend.pop(0)
                            emit_nd(b, h, ptk, peT)
                        setup_piece(b, nxt, h, tk)
                    emit_nd_flush(b, h)
                # batch 3 finale runs inline at the very end
                if nxt is None:
                    for stp in range(4):
                        emit_finale(b, stp)

            # single combined output DMA
            nc.sync.dma_start(
                out_d[:, :].rearrange("b (t p) -> t b p", p=128),
                o16.rearrange("t (b p) -> t b p", p=128))

    nc.compile()
    return nc


def run(inputs: dict, trace: bool = False):
    _install_axon_shim()
    import ml_dtypes
    from concourse.bass_utils import run_bass_kernel_spmd

    z = np.asarray(inputs["z"], dtype=np.float32)
    Wq = np.asarray(inputs["Wq"], dtype=np.float64)
    bq = np.asarray(inputs["bq"], dtype=np.float64)
    Wk = np.asarray(inputs["Wk"], dtype=np.float64)
    Wv = np.asarray(inputs["Wv"], dtype=np.float64)
    bv = np.asarray(inputs["bv"], dtype=np.float64)
    Wo = np.asarray(inputs["Wo"], dtype=np.float64)
    bo = np.asarray(inputs["bo"], dtype=np.float64)

    # host-side weight algebra (tiny, exact in float64)
    m_lhs = (Wq.T @ Wk).astype(np.float32)            # [d, d']
    gw = (Wk.T @ bq).astype(np.float32).reshape(D, 1)
    wv = (Wv.T @ Wo[0]).astype(np.float32)            # [d]
    wvb = np.broadcast_to(np.tile(wv, A // D), (128, A)).astype(
        ml_dtypes.bfloat16)
    cbo_val = float(bv @ Wo[0] + bo[0])

    z_bf = z.astype(ml_dtypes.bfloat16)

    nc = _build_program(cbo_val)

    in_maps = []
    for c in range(N_CORES):
        in_maps.append({
            "z": z_bf[c * B_PER:(c + 1) * B_PER],
            "m_lhs": m_lhs,
            "gw": gw,
            "wvb": np.ascontiguousarray(wvb),
        })
    res = run_bass_kernel_spmd(nc, in_maps, core_ids=list(range(N_CORES)),
                               trace=trace)
    out = np.concatenate([res.results[c]["out"] for c in range(N_CORES)],
                         axis=0)
    return out.reshape(B, A, 1).astype(np.float32), res


def kernel(**inputs) -> np.ndarray:
    out, _ = run(inputs, trace=False)
    return out
